# revision 5
# baseline (speedup 1.0000x reference)
"""Causal multi-head attention block (B=2, L=2048, D=1024, H=16) on 8 trn2 cores.

Sharding: core c -> batch b = c // 4, head group g = c % 4 (heads 4g..4g+4).
All matmul operands bf16 (fp32 PSUM accumulation); inputs pre-cast on host.
Per core:
  1. QT/KT = (W_qk x^T)          (d_head on partitions; 512 x 2048 each core)
  2. V     = (x W_v^T + b_v)     (j on partitions; two ones-columns per head
                                  accumulate the softmax denominators)
  3. per (head-pair, 512-query-block): ST = K Q^T for both heads (row-tiled
     64-deep matmuls run concurrently), causal-narrowed diagonal chunks;
     P^T = exp(0.125 ST) in one wide ACT call covering both heads; the
     128-col diagonal wedge is zeroed by a 0/1-triangle DVE multiply;
     O^T|l = [V|1]^T P^T per head; normalization: broadcast l by a selector
     matmul, reciprocal_approx_fast, one DVE multiply per head.
  4. y^T_partial = W_out,local O^T (PSUM -> SBUF bf16 -> DMA)
Host: y[b] = sum of the 4 partials^T + b_out.
Projection / next-block QKV matmuls are interleaved between attention pairs to
keep the PE busy (HAM stays at full clock).
"""

import numpy as np
import ml_dtypes

import concourse.bass as bass
import concourse.bacc as bacc
import concourse.mybir as mybir
from concourse.tile import TileContext
from concourse.bass_utils import run_bass_kernel_spmd

B, L, D, H = 2, 2048, 1024, 16
HD = 64                      # head dim
HPC = 4                      # heads per core
DL = HPC * HD                # 256 local head dims
N_CORES = 8
SCALE = 1.0 / 8.0            # 1/sqrt(64)
FP32 = mybir.dt.float32
FP32R = mybir.dt.float32r
BF16 = mybir.dt.bfloat16
AF = mybir.ActivationFunctionType
BFNP = ml_dtypes.bfloat16

NKC = D // 128               # 8 contraction chunks over D
NMB = L // 512               # 4 column blocks of 512 over L
NJC = L // 128               # 16 j-chunks of 128


def r32(ap):
    return ap.bitcast(FP32R)


def bf(a):
    return np.ascontiguousarray(np.asarray(a, np.float32)).astype(BFNP)


def build_program():
    nc = bacc.Bacc("TRN2", target_bir_lowering=False, debug=False)

    xt = nc.dram_tensor("xt", [D, L], BF16, kind="ExternalInput")
    wqk = nc.dram_tensor("wqk", [D, 2 * DL], BF16, kind="ExternalInput")
    wv = nc.dram_tensor("wv", [D, DL], BF16, kind="ExternalInput")
    wout = nc.dram_tensor("wout", [DL, D], BF16, kind="ExternalInput")
    bqk = nc.dram_tensor("bqk", [2 * DL, 1], FP32, kind="ExternalInput")
    bv = nc.dram_tensor("bv", [1, DL], FP32, kind="ExternalInput")
    trid = nc.dram_tensor("trid", [128, 128], BF16, kind="ExternalInput")
    seld = nc.dram_tensor("seld", [34, 128], FP32, kind="ExternalInput")
    yt = nc.dram_tensor("yt", [D, L], BF16, kind="ExternalOutput")

    with TileContext(nc) as tc:
        with (
            tc.tile_pool(name="const", bufs=1) as const,
            tc.tile_pool(name="xtp", bufs=12) as xtp,
            tc.tile_pool(name="ptp", bufs=2) as ptp,
            tc.tile_pool(name="rp", bufs=2) as rp,
            tc.tile_pool(name="yp", bufs=2) as yp,
            tc.tile_pool(name="ps_st", bufs=1, space="PSUM") as ps_st,
            tc.tile_pool(name="ps_ot", bufs=2, space="PSUM") as ps_ot,
            tc.tile_pool(name="ps_sm", bufs=2, space="PSUM") as ps_sm,
        ):
            # ---- persistent constants / weights ----
            # (DMA issue order matters: first qk-unit needs wqk + bqk + x block 0,
            # so those are queued first; the rest follows behind x block 0/1.)
            wqk_t = []
            xts0 = []
            for kc in range(NKC):
                t = const.tile([128, 2 * DL], BF16, tag=f"wqk{kc}")
                nc.sync.dma_start(out=t[:], in_=wqk[kc * 128:(kc + 1) * 128, :])
                wqk_t.append(t)
                tx = xtp.tile([128, 512], BF16, name="t")
                nc.sync.dma_start(
                    out=tx[:], in_=xt[kc * 128:(kc + 1) * 128, 0:512])
                xts0.append(tx)
            bq_t = []
            for nt in range(4):
                t = const.tile([128, 1], FP32, tag=f"bqk{nt}")
                nc.sync.dma_start(out=t[:], in_=bqk[nt * 128:(nt + 1) * 128, :])
                bq_t.append(t)

            def load_consts2():
                wv_t = []
                for kc in range(NKC):
                    t = const.tile([128, DL], BF16, tag=f"wv{kc}")
                    nc.sync.dma_start(out=t[:],
                                      in_=wv[kc * 128:(kc + 1) * 128, :])
                    wv_t.append(t)
                bvrep = const.tile([128, DL], FP32, tag="bvrep")
                nc.sync.dma_start(out=bvrep[:],
                                  in_=bv[0:1, :].to_broadcast((128, DL)))
                sel_t = const.tile([34, 128], FP32R, tag="sel")
                nc.sync.dma_start(out=sel_t[:], in_=r32(seld[:, :]))
                tri_t = const.tile([128, 128], BF16, tag="tri")
                nc.gpsimd.dma_start(out=tri_t[:], in_=trid[:, :])
                return wv_t, bvrep, sel_t, tri_t

            def load_consts3():
                wout_t = []
                for n2 in range(2):
                    t = const.tile([128, D], BF16, tag=f"wout{n2}")
                    nc.sync.dma_start(out=t[:],
                                      in_=wout[n2 * 128:(n2 + 1) * 128, :])
                    wout_t.append(t)
                return wout_t

            # persistent activations
            # qk_t[0..1]: QT tiles (128 rows each: heads {2i,2i+1}); qk_t[2..3]: KT
            qk_t = [const.tile([128, L], BF16, tag=f"qk{nt}", name=f"qk{nt}")
                    for nt in range(4)]
            # V tiles per j-chunk: [128, 4*66]; head h cols h*66..h*66+64 = V,
            # cols h*66+64..66 = 1.0 (denominator accumulator columns)
            v_t = [const.tile([128, 4 * 66], BF16, tag=f"v{j}", name=f"v{j}")
                   for j in range(NJC)]
            ot_t = [const.tile([128, L], BF16, tag=f"ot{n2}", name=f"ot{n2}")
                    for n2 in range(2)]
            # persistent denominator staging rows (only rows 0 and 32 written)
            ls_t = const.tile([34, 512], FP32R, tag="ls")
            nc.vector.memset(ls_t[:].bitcast(FP32), 0.0)

            def load_x(m):
                xts = []
                for kc in range(NKC):
                    t = xtp.tile([128, 512], BF16)
                    nc.sync.dma_start(
                        out=t[:],
                        in_=xt[kc * 128:(kc + 1) * 128, m * 512:(m + 1) * 512])
                    xts.append(t)
                return xts

            def qk_unit(xts, m, nt):
                ps = ps_sm.tile([128, 512], FP32, tag="ps_sm")
                for kc in range(NKC):
                    nc.tensor.matmul(
                        ps[:],
                        wqk_t[kc][:, nt * 128:(nt + 1) * 128],
                        xts[kc][:],
                        start=(kc == 0), stop=(kc == NKC - 1))
                with nc.allow_low_precision(reason="bf16 activations"):
                    nc.vector.tensor_scalar_add(
                        qk_t[nt][:, m * 512:(m + 1) * 512], ps[:], bq_t[nt][:])

            def v_unit(xts, m, ic):
                j = 4 * m + ic
                ps = ps_sm.tile([128, 512], FP32, tag="ps_sm")
                for kc in range(NKC):
                    nc.tensor.matmul(
                        ps[:, 0:DL],
                        xts[kc][:, ic * 128:(ic + 1) * 128],
                        wv_t[kc][:],
                        start=(kc == 0), stop=(kc == NKC - 1))
                v4 = v_t[j][:].rearrange("p (h m) -> p h m", m=66)
                with nc.allow_low_precision(reason="bf16 activations"):
                    nc.vector.tensor_add(
                        v4[:, :, 0:64],
                        ps[:, 0:DL].rearrange("p (h d) -> p h d", d=64),
                        bvrep[:].rearrange("p (h d) -> p h d", d=64))
                nc.gpsimd.memset(v4[:, :, 64:66], 1.0)

            def attn_pair(hs, t_, filler):
                """Two heads' ST -> exp -> PV chains; the two heads' 64-deep
                score matmuls run concurrently as PE row-tiles."""
                n_j = 4 * (t_ + 1)
                hp = hs[0] // 2
                qt = qk_t[hp]
                kt = qk_t[2 + hp]
                otps = [ps_ot.tile([128, 512], FP32, tag="ps_ot", name=f"otp{h}")
                        for h in hs]
                for jp in range(0, n_j, 2):
                    # stp cols: h-half*1024 + jj*512 + i
                    stp = ps_st.tile([128, 2048], FP32, tag="ps_st")
                    ws = []
                    for jj in range(2):
                        J = jp + jj
                        q = J - 4 * t_      # >= 0 on the diagonal band
                        w0 = 128 * q if q > 0 else 0
                        ws.append(w0)
                        for i, h in enumerate(hs):
                            po = (h % 2) * 64
                            nc.tensor.matmul(
                                stp[:, i * 1024 + jj * 512 + w0:
                                    i * 1024 + (jj + 1) * 512],
                                kt[po:po + 64, J * 128:(J + 1) * 128],
                                qt[po:po + 64, t_ * 512 + w0:(t_ + 1) * 512],
                                start=True, stop=True)
                    ptile = ptp.tile([128, 2048], BF16, name="pt")
                    st3 = stp[:].rearrange("p (h i) -> p h i", h=2)
                    pt3 = ptile[:].rearrange("p (h i) -> p h i", h=2)
                    diag = jp >= 4 * t_
                    with nc.allow_low_precision(reason="bf16 probs"):
                        if diag:
                            for jj in range(2):
                                sl = slice(jj * 512 + ws[jj], (jj + 1) * 512)
                                nc.scalar.activation(
                                    pt3[:, :, sl], st3[:, :, sl],
                                    AF.Exp, scale=SCALE)
                            # zero the in-chunk causal wedge (first 128 cols
                            # of each narrowed slice): keep p <= col-offset
                            for jj in range(2):
                                w0 = ws[jj]
                                for i in range(2):
                                    o = i * 1024 + jj * 512 + w0
                                    nc.vector.tensor_mul(
                                        ptile[:, o:o + 128],
                                        ptile[:, o:o + 128], tri_t[:])
                        else:
                            nc.scalar.activation(ptile[:], stp[:],
                                                 AF.Exp, scale=SCALE)
                    filler()
                    for jj in range(2):
                        J = jp + jj
                        w0 = ws[jj]
                        for i, h in enumerate(hs):
                            nc.tensor.matmul(
                                otps[i][0:66, w0:512],
                                v_t[J][:, h * 66:(h + 1) * 66],
                                ptile[:, i * 1024 + jj * 512 + w0:
                                      i * 1024 + (jj + 1) * 512],
                                start=(J == 0), stop=(J == n_j - 1))
                return otps

            def norm_stage(otps):
                """Copy both heads' O^T out of PSUM (releases PV slots), stack
                the denominator rows for the selector matmul."""
                osb = rp.tile([128, 512], FP32, name="osb")
                for i, otp in enumerate(otps):
                    nc.vector.tensor_copy(osb[64 * i:64 * i + 64, :],
                                          otp[0:64, :])
                    with nc.allow_low_precision(reason="fp32r matmul operand"):
                        nc.vector.tensor_copy(ls_t[32 * i:32 * i + 1, :],
                                              otp[64:65, :])
                return osb

            def norm_finish(osb, hs, t_):
                """Broadcast denominators by matmul, fast-reciprocal, scale
                into ot_t. Deferred into the next pair's filler slot."""
                isl = slice(t_ * 512, (t_ + 1) * 512)
                rb = ps_sm.tile([128, 512], FP32, tag="ps_sm")
                nc.tensor.matmul(rb[:], sel_t[:], ls_t[:],
                                 start=True, stop=True)
                rbb = rp.tile([128, 512], FP32, name="rbb")
                nc.vector.reciprocal_approx_fast(out=rbb[:], in_=rb[:])
                for i, h in enumerate(hs):
                    with nc.allow_low_precision(reason="bf16 activations"):
                        nc.vector.tensor_mul(
                            ot_t[h // 2][(h % 2) * 64:(h % 2) * 64 + 64, isl],
                            osb[64 * i:64 * i + 64, :],
                            rbb[64 * i:64 * i + 64, :])

            def proj_unit(t_, dt_):
                isl = slice(t_ * 512, (t_ + 1) * 512)
                ps = ps_sm.tile([128, 512], FP32, tag="ps_sm")
                for n2 in range(2):
                    nc.tensor.matmul(
                        ps[:],
                        wout_t[n2][:, dt_ * 128:(dt_ + 1) * 128],
                        ot_t[n2][:, isl],
                        start=(n2 == 0), stop=(n2 == 1))
                ys = yp.tile([128, 512], BF16, name="ys")
                with nc.allow_low_precision(reason="bf16 partial output"):
                    nc.vector.tensor_copy(ys[:], ps[:])
                nc.sync.dma_start(
                    out=yt[dt_ * 128:(dt_ + 1) * 128, isl], in_=ys[:])

            # ---- program ----
            wv_t, bvrep, sel_t, tri_t = load_consts2()
            for u in range(4):
                qk_unit(xts0, 0, u)
                v_unit(xts0, 0, u)
            xts1 = load_x(1)
            wout_t = load_consts3()
            for u in range(4):
                qk_unit(xts1, 1, u)
                v_unit(xts1, 1, u)

            # attention block order [0,3,1,2] balances filler-unit supply:
            # t=0 gets both remaining qkv blocks, t=3 gets proj(0), t=1 gets
            # proj(3), t=2 gets proj(1); proj(2) trails densely at the end.
            unit_plan = {0: [], 3: [], 1: [], 2: []}
            for t_ in (0, 3, 1, 2):
                units = unit_plan[t_]
                if t_ == 0:
                    for m in (2, 3):
                        xts = load_x(m)
                        for u in range(4):
                            units.append(lambda u=u, xts=xts, m=m:
                                         qk_unit(xts, m, u))
                            units.append(lambda u=u, xts=xts, m=m:
                                         v_unit(xts, m, u))
                else:
                    pm = {3: 0, 1: 3, 2: 1}[t_]
                    for dt_ in range(8):
                        units.append(lambda dt_=dt_, m=pm: proj_unit(m, dt_))

                n_slots = 4 * (t_ + 1) + 2
                # dispatch units in bursts of 2 (~4us dense PE work) so the
                # HAM activity window can re-warm the clock
                burst_every = max(1, (2 * n_slots) // max(1, len(units)))
                state = {"i": 0, "slot": 0}

                def filler(state=state, units=units, burst_every=burst_every):
                    if state["slot"] % burst_every == 0 and state["i"] < len(units):
                        units[state["i"]]()
                        state["i"] += 1
                        if state["i"] < len(units):
                            units[state["i"]]()
                            state["i"] += 1
                    else:
                        pass
                    state["slot"] += 1

                def filler2(state=state):
                    if state.get("pending") is not None:
                        fin = state["pending"]
                        state["pending"] = None
                        fin()
                    filler()

                for hp in range(2):
                    hs = (2 * hp, 2 * hp + 1)
                    otps = attn_pair(hs, t_, filler2)
                    osb = norm_stage(otps)
                    state["pending"] = (
                        lambda osb=osb, hs=hs: norm_finish(osb, hs, t_))
                    filler()
                if state.get("pending") is not None:
                    state["pending"]()
                    state["pending"] = None
                while state["i"] < len(units):
                    units[state["i"]]()
                    state["i"] += 1

            for dt_ in range(8):
                proj_unit(2, dt_)

    nc.compile()
    return nc


_NC_CACHE = None


def _get_nc():
    global _NC_CACHE
    if _NC_CACHE is None:
        _NC_CACHE = build_program()
    return _NC_CACHE


def make_in_maps(x, W_qkv, b_qkv, W_out):
    """Per-core input dicts (core c -> batch c//4, head group c%4)."""
    p_ = np.arange(128)[:, None]
    c_ = np.arange(128)[None, :]
    trid = (p_ <= c_).astype(np.float32)   # keep jp <= i-offset (causal)

    in_maps = []
    for c in range(N_CORES):
        b, g = divmod(c, 4)
        rs = slice(DL * g, DL * g + DL)
        wq = W_qkv[0 * D:1 * D][rs]
        wk = W_qkv[1 * D:2 * D][rs]
        wv = W_qkv[2 * D:3 * D][rs]
        in_maps.append({
            "xt": bf(x[b].T),
            "wqk": bf(np.concatenate([wq, wk], 0).T),
            "wv": bf(wv.T),
            "wout": bf(W_out[:, rs].T),
            "bqk": np.ascontiguousarray(
                np.concatenate([b_qkv[0 * D:1 * D][rs],
                                b_qkv[1 * D:2 * D][rs]])[:, None], np.float32),
            "bv": np.ascontiguousarray(b_qkv[2 * D:3 * D][rs][None, :], np.float32),
            "trid": bf(trid),
            "seld": np.concatenate([
                np.repeat(np.eye(2, dtype=np.float32), 64, axis=1)[0:1],
                np.zeros((31, 128), np.float32),
                np.repeat(np.eye(2, dtype=np.float32), 64, axis=1)[1:2],
                np.zeros((1, 128), np.float32)]),
        })
    return in_maps


def assemble_output(results, b_out):
    y = np.zeros((B, L, D), np.float32)
    for c in range(N_CORES):
        b = c // 4
        y[b] += np.asarray(results[c]["yt"], np.float32).T
    y += b_out[None, None, :].astype(np.float32)
    return y


def run(x, mask, W_qkv, b_qkv, W_out, b_out, trace=False, **spmd_kwargs):
    causal = np.array_equal(
        np.asarray(mask).reshape(L, L),
        np.triu(np.ones((L, L), bool), k=1))
    if not causal:
        # Fallback (never expected): reference semantics on host.
        print("WARNING: non-causal mask; computing on host")
        q, k, v = np.split(x @ W_qkv.T + b_qkv, 3, axis=-1)
        th = lambda t: t.reshape(B, L, H, HD).transpose(0, 2, 1, 3)
        q, k, v = th(q), th(k), th(v)
        a = np.einsum('bhqd,bhkd->bhqk', q, k) * SCALE
        a = np.where(np.asarray(mask), -np.inf, a)
        a = a - a.max(-1, keepdims=True)
        a = np.exp(a)
        a /= a.sum(-1, keepdims=True)
        o = np.einsum('bhqk,bhkd->bhqd', a, v)
        o = o.transpose(0, 2, 1, 3).reshape(B, L, D)
        return o @ W_out.T + b_out, None

    nc = _get_nc()
    in_maps = make_in_maps(np.asarray(x), np.asarray(W_qkv),
                           np.asarray(b_qkv), np.asarray(W_out))
    res = run_bass_kernel_spmd(nc, in_maps, list(range(N_CORES)),
                               trace=trace, **spmd_kwargs)
    y = assemble_output(res.results, np.asarray(b_out))
    return y, res


def kernel(x, mask, W_qkv, b_qkv, W_out, b_out):
    y, _ = run(x, mask, W_qkv, b_qkv, W_out, b_out)
    return y


# revision 9
# speedup vs baseline: 1.4932x; 1.4932x over previous
"""Causal multi-head attention block (B=2, L=2048, D=1024, H=16) on 8 trn2 cores.

Sharding: core c -> batch b = c // 4, head group g = c % 4 (heads 4g..4g+4).
All matmul operands bf16 (fp32 PSUM accumulation); inputs pre-cast on host.
Per core:
  1. QT/KT = (W_qk x^T)          (d_head on partitions; 512 x 2048 each core)
  2. V     = (x W_v^T + b_v)     (j on partitions; two ones-columns per head
                                  accumulate the softmax denominators)
  3. per (head-pair, 512-query-block): ST = K Q^T for both heads (row-tiled
     64-deep matmuls run concurrently), causal-narrowed diagonal chunks;
     P^T = exp(0.125 ST) in one wide ACT call covering both heads; the
     128-col diagonal wedge is zeroed by a 0/1-triangle DVE multiply;
     O^T|l = [V|1]^T P^T per head; normalization: broadcast l by a selector
     matmul, reciprocal_approx_fast, one DVE multiply per head.
  4. y^T_partial = W_out,local O^T (PSUM -> SBUF bf16 -> DMA)
Host: y[b] = sum of the 4 partials^T + b_out.
Projection / next-block QKV matmuls are interleaved between attention pairs to
keep the PE busy (HAM stays at full clock).
"""

import numpy as np
import ml_dtypes

import concourse.bass as bass
import concourse.bacc as bacc
import concourse.mybir as mybir
from concourse.tile import TileContext
from concourse.bass_utils import run_bass_kernel_spmd

B, L, D, H = 2, 2048, 1024, 16
HD = 64                      # head dim
HPC = 4                      # heads per core
DL = HPC * HD                # 256 local head dims
N_CORES = 8
SCALE = 1.0 / 8.0            # 1/sqrt(64)
FP32 = mybir.dt.float32
FP32R = mybir.dt.float32r
BF16 = mybir.dt.bfloat16
AF = mybir.ActivationFunctionType
BFNP = ml_dtypes.bfloat16

NKC = D // 128               # 8 contraction chunks over D
NMB = L // 512               # 4 column blocks of 512 over L
NJC = L // 128               # 16 j-chunks of 128


def r32(ap):
    return ap.bitcast(FP32R)


def bf(a):
    return np.ascontiguousarray(np.asarray(a, np.float32)).astype(BFNP)


def build_program():
    nc = bacc.Bacc("TRN2", target_bir_lowering=False, debug=False)

    xt = nc.dram_tensor("xt", [D, L], BF16, kind="ExternalInput")
    wqk = nc.dram_tensor("wqk", [D, 2 * DL], BF16, kind="ExternalInput")
    wv = nc.dram_tensor("wv", [D, DL], BF16, kind="ExternalInput")
    wout = nc.dram_tensor("wout", [DL, D], BF16, kind="ExternalInput")
    bqk = nc.dram_tensor("bqk", [2 * DL, 1], FP32, kind="ExternalInput")
    bv = nc.dram_tensor("bv", [1, DL], FP32, kind="ExternalInput")
    trid = nc.dram_tensor("trid", [128, 128], BF16, kind="ExternalInput")
    seld = nc.dram_tensor("seld", [34, 128], FP32, kind="ExternalInput")
    yt = nc.dram_tensor("yt", [D, L], BF16, kind="ExternalOutput")

    with TileContext(nc) as tc:
        with (
            tc.tile_pool(name="const", bufs=1) as const,
            tc.tile_pool(name="xtp", bufs=12) as xtp,
            tc.tile_pool(name="ptp", bufs=4) as ptp,
            tc.tile_pool(name="rp", bufs=2) as rp,
            tc.tile_pool(name="yp", bufs=2) as yp,
            tc.tile_pool(name="ps_st", bufs=2, space="PSUM") as ps_st,
            tc.tile_pool(name="ps_ot", bufs=2, space="PSUM") as ps_ot,
            tc.tile_pool(name="ps_sm", bufs=2, space="PSUM") as ps_sm,
        ):
            # ---- persistent constants / weights ----
            # (DMA issue order matters: first qk-unit needs wqk + bqk + x block 0,
            # so those are queued first; the rest follows behind x block 0/1.)
            wqk_t = []
            xts0 = []
            for kc in range(NKC):
                t = const.tile([128, 2 * DL], BF16, tag=f"wqk{kc}")
                nc.sync.dma_start(out=t[:], in_=wqk[kc * 128:(kc + 1) * 128, :])
                wqk_t.append(t)
                tx = xtp.tile([128, 512], BF16, name="t")
                nc.sync.dma_start(
                    out=tx[:], in_=xt[kc * 128:(kc + 1) * 128, 0:512])
                xts0.append(tx)
            bq_t = []
            for nt in range(4):
                t = const.tile([128, 1], FP32, tag=f"bqk{nt}")
                nc.sync.dma_start(out=t[:], in_=bqk[nt * 128:(nt + 1) * 128, :])
                bq_t.append(t)

            def load_consts2():
                wv_t = []
                for kc in range(NKC):
                    t = const.tile([128, DL], BF16, tag=f"wv{kc}")
                    nc.sync.dma_start(out=t[:],
                                      in_=wv[kc * 128:(kc + 1) * 128, :])
                    wv_t.append(t)
                bvrep = const.tile([128, DL], FP32, tag="bvrep")
                nc.sync.dma_start(out=bvrep[:],
                                  in_=bv[0:1, :].to_broadcast((128, DL)))
                sel_t = const.tile([34, 128], FP32R, tag="sel")
                nc.sync.dma_start(out=sel_t[:], in_=r32(seld[:, :]))
                tri_t = const.tile([128, 128], BF16, tag="tri")
                nc.gpsimd.dma_start(out=tri_t[:], in_=trid[:, :])
                return wv_t, bvrep, sel_t, tri_t

            def load_consts3():
                wout_t = []
                for n2 in range(2):
                    t = const.tile([128, D], BF16, tag=f"wout{n2}")
                    nc.sync.dma_start(out=t[:],
                                      in_=wout[n2 * 128:(n2 + 1) * 128, :])
                    wout_t.append(t)
                return wout_t

            # persistent activations
            # qk_t[0..1]: QT tiles (128 rows each: heads {2i,2i+1}); qk_t[2..3]: KT
            qk_t = [const.tile([128, L], BF16, tag=f"qk{nt}", name=f"qk{nt}")
                    for nt in range(4)]
            # V tiles per j-chunk: [128, 4*66]; head h cols h*66..h*66+64 = V,
            # cols h*66+64..66 = 1.0 (denominator accumulator columns)
            v_t = [const.tile([128, 4 * 66], BF16, tag=f"v{j}", name=f"v{j}")
                   for j in range(NJC)]
            ot_t = [const.tile([128, L], BF16, tag=f"ot{n2}", name=f"ot{n2}")
                    for n2 in range(2)]
            # persistent denominator staging rows (only rows 0 and 32 written)
            ls_t = const.tile([34, 512], FP32R, tag="ls")
            nc.vector.memset(ls_t[:].bitcast(FP32), 0.0)

            def load_x(m):
                xts = []
                for kc in range(NKC):
                    t = xtp.tile([128, 512], BF16)
                    nc.sync.dma_start(
                        out=t[:],
                        in_=xt[kc * 128:(kc + 1) * 128, m * 512:(m + 1) * 512])
                    xts.append(t)
                return xts

            def qk_unit(xts, m, nt):
                ps = ps_sm.tile([128, 512], FP32, tag="ps_sm")
                for kc in range(NKC):
                    nc.tensor.matmul(
                        ps[:],
                        wqk_t[kc][:, nt * 128:(nt + 1) * 128],
                        xts[kc][:],
                        start=(kc == 0), stop=(kc == NKC - 1))
                with nc.allow_low_precision(reason="bf16 activations"):
                    nc.vector.tensor_scalar_add(
                        qk_t[nt][:, m * 512:(m + 1) * 512], ps[:], bq_t[nt][:])

            def v_unit(xts, m, ic):
                j = 4 * m + ic
                ps = ps_sm.tile([128, 512], FP32, tag="ps_sm")
                for kc in range(NKC):
                    nc.tensor.matmul(
                        ps[:, 0:DL],
                        xts[kc][:, ic * 128:(ic + 1) * 128],
                        wv_t[kc][:],
                        start=(kc == 0), stop=(kc == NKC - 1))
                v4 = v_t[j][:].rearrange("p (h m) -> p h m", m=66)
                with nc.allow_low_precision(reason="bf16 activations"):
                    nc.vector.tensor_add(
                        v4[:, :, 0:64],
                        ps[:, 0:DL].rearrange("p (h d) -> p h d", d=64),
                        bvrep[:].rearrange("p (h d) -> p h d", d=64))
                nc.gpsimd.memset(v4[:, :, 64:66], 1.0)

            def attn_pair(hs, t_, filler):
                """Two heads' ST -> exp -> PV chains, software-pipelined: PV
                for slot k-1 issues while slot k's exp runs, so the in-order
                PE queue never stalls on the scalar engine. The two heads'
                64-deep score matmuls run concurrently as PE row-tiles; each
                head has its own PSUM score tile so head A's next scores can
                start as soon as head A's exp has drained."""
                n_j = 4 * (t_ + 1)
                hp = hs[0] // 2
                qt = qk_t[hp]
                kt = qk_t[2 + hp]
                otps = [ps_ot.tile([128, 512], FP32, tag="ps_ot", name=f"otp{h}")
                        for h in hs]

                def pv(prev):
                    ptiles, jp0, ws = prev
                    for jj in range(2):
                        J = jp0 + jj
                        w0 = ws[jj]
                        for i, h in enumerate(hs):
                            nc.tensor.matmul(
                                otps[i][0:66, w0:512],
                                v_t[J][:, h * 66:(h + 1) * 66],
                                ptiles[i][:, jj * 512 + w0:(jj + 1) * 512],
                                start=(J == 0), stop=(J == n_j - 1))

                prev = None
                for jp in range(0, n_j, 2):
                    ws = []
                    stps = []
                    for jj in range(2):
                        J = jp + jj
                        q = J - 4 * t_      # >= 0 on the diagonal band
                        ws.append(128 * q if q > 0 else 0)
                    for i, h in enumerate(hs):
                        po = (h % 2) * 64
                        stp = ps_st.tile([128, 1024], FP32, tag="ps_st",
                                         name=f"stp{h}")
                        for jj in range(2):
                            J = jp + jj
                            # full width even on diagonal chunks: the w0
                            # leading columns are fully masked garbage that
                            # exp computes and PV skips (keeps PSUM fully
                            # initialized and call shapes uniform)
                            nc.tensor.matmul(
                                stp[:, jj * 512:(jj + 1) * 512],
                                kt[po:po + 64, J * 128:(J + 1) * 128],
                                qt[po:po + 64, t_ * 512:(t_ + 1) * 512],
                                start=True, stop=True)
                        stps.append(stp)
                    ptiles = []
                    for i, h in enumerate(hs):
                        ptile = ptp.tile([128, 1024], BF16, name=f"pt{h}")
                        # full-width exp: the w0-leading garbage columns are
                        # never read by PV; uniform calls keep ACT saturated
                        with nc.allow_low_precision(reason="bf16 probs"):
                            nc.scalar.activation(ptile[:], stps[i][:],
                                                 AF.Exp, scale=SCALE)
                        ptiles.append(ptile)
                    if jp >= 4 * t_:
                        # zero the in-chunk causal wedge (first 128 columns
                        # of each diagonal chunk): keep p <= col-offset
                        for i in range(2):
                            for jj in range(2):
                                o = jj * 512 + ws[jj]
                                nc.gpsimd.tensor_mul(
                                    ptiles[i][:, o:o + 128],
                                    ptiles[i][:, o:o + 128], tri_t[:])
                    filler()
                    if prev is not None:
                        pv(prev)
                    prev = (ptiles, jp, ws)
                pv(prev)
                return otps

            def norm_stage(otps):
                """Copy both heads' O^T out of PSUM (releases PV slots), stack
                the denominator rows for the selector matmul."""
                osb = rp.tile([128, 512], FP32, name="osb")
                for i, otp in enumerate(otps):
                    nc.vector.tensor_copy(osb[64 * i:64 * i + 64, :],
                                          otp[0:64, :])
                    with nc.allow_low_precision(reason="fp32r matmul operand"):
                        nc.vector.tensor_copy(ls_t[32 * i:32 * i + 1, :],
                                              otp[64:65, :])
                return osb

            def norm_finish(osb, hs, t_):
                """Broadcast denominators by matmul, fast-reciprocal, scale
                into ot_t. Deferred into the next pair's filler slot."""
                isl = slice(t_ * 512, (t_ + 1) * 512)
                rb = ps_sm.tile([128, 512], FP32, tag="ps_sm")
                nc.tensor.matmul(rb[:], sel_t[:], ls_t[:],
                                 start=True, stop=True)
                rbb = rp.tile([128, 512], FP32, name="rbb")
                nc.vector.reciprocal_approx_fast(out=rbb[:], in_=rb[:])
                for i, h in enumerate(hs):
                    with nc.allow_low_precision(reason="bf16 activations"):
                        nc.vector.tensor_mul(
                            ot_t[h // 2][(h % 2) * 64:(h % 2) * 64 + 64, isl],
                            osb[64 * i:64 * i + 64, :],
                            rbb[64 * i:64 * i + 64, :])

            def proj_unit(t_, dt_):
                isl = slice(t_ * 512, (t_ + 1) * 512)
                ps = ps_sm.tile([128, 512], FP32, tag="ps_sm")
                for n2 in range(2):
                    nc.tensor.matmul(
                        ps[:],
                        wout_t[n2][:, dt_ * 128:(dt_ + 1) * 128],
                        ot_t[n2][:, isl],
                        start=(n2 == 0), stop=(n2 == 1))
                ys = yp.tile([128, 512], BF16, name="ys")
                with nc.allow_low_precision(reason="bf16 partial output"):
                    nc.vector.tensor_copy(ys[:], ps[:])
                nc.sync.dma_start(
                    out=yt[dt_ * 128:(dt_ + 1) * 128, isl], in_=ys[:])

            # ---- program ----
            wv_t, bvrep, sel_t, tri_t = load_consts2()
            for u in range(4):
                qk_unit(xts0, 0, u)
                v_unit(xts0, 0, u)
            xts1 = load_x(1)
            wout_t = load_consts3()
            for u in range(4):
                qk_unit(xts1, 1, u)
                v_unit(xts1, 1, u)

            # attention block order [0,3,1,2] balances filler-unit supply:
            # t=0 gets both remaining qkv blocks, t=3 gets proj(0), t=1 gets
            # proj(3), t=2 gets proj(1); proj(2) trails densely at the end.
            unit_plan = {0: [], 3: [], 1: [], 2: []}
            for t_ in (0, 3, 1, 2):
                units = unit_plan[t_]
                if t_ == 0:
                    for m in (2, 3):
                        xts = load_x(m)
                        for u in range(4):
                            units.append(lambda u=u, xts=xts, m=m:
                                         qk_unit(xts, m, u))
                            units.append(lambda u=u, xts=xts, m=m:
                                         v_unit(xts, m, u))
                else:
                    pm = {3: 0, 1: 3, 2: 1}[t_]
                    for dt_ in range(8):
                        units.append(lambda dt_=dt_, m=pm: proj_unit(m, dt_))

                n_slots = 4 * (t_ + 1) + 2
                # dispatch units in bursts of 2 (~4us dense PE work) so the
                # HAM activity window can re-warm the clock
                burst_every = max(1, (2 * n_slots) // max(1, len(units)))
                state = {"i": 0, "slot": 0}

                def filler(state=state, units=units, burst_every=burst_every):
                    if state["slot"] % burst_every == 0 and state["i"] < len(units):
                        units[state["i"]]()
                        state["i"] += 1
                        if state["i"] < len(units):
                            units[state["i"]]()
                            state["i"] += 1
                    else:
                        pass
                    state["slot"] += 1

                def filler2(state=state):
                    if state.get("pending") is not None:
                        fin = state["pending"]
                        state["pending"] = None
                        fin()
                    filler()

                for hp in range(2):
                    hs = (2 * hp, 2 * hp + 1)
                    otps = attn_pair(hs, t_, filler2)
                    osb = norm_stage(otps)
                    state["pending"] = (
                        lambda osb=osb, hs=hs: norm_finish(osb, hs, t_))
                    filler()
                if state.get("pending") is not None:
                    state["pending"]()
                    state["pending"] = None
                while state["i"] < len(units):
                    units[state["i"]]()
                    state["i"] += 1

            for dt_ in range(8):
                proj_unit(2, dt_)

    nc.compile()
    return nc


_NC_CACHE = None


def _get_nc():
    global _NC_CACHE
    if _NC_CACHE is None:
        _NC_CACHE = build_program()
    return _NC_CACHE


def make_in_maps(x, W_qkv, b_qkv, W_out):
    """Per-core input dicts (core c -> batch c//4, head group c%4)."""
    p_ = np.arange(128)[:, None]
    c_ = np.arange(128)[None, :]
    trid = (p_ <= c_).astype(np.float32)   # keep jp <= i-offset (causal)

    in_maps = []
    for c in range(N_CORES):
        b, g = divmod(c, 4)
        rs = slice(DL * g, DL * g + DL)
        wq = W_qkv[0 * D:1 * D][rs]
        wk = W_qkv[1 * D:2 * D][rs]
        wv = W_qkv[2 * D:3 * D][rs]
        in_maps.append({
            "xt": bf(x[b].T),
            "wqk": bf(np.concatenate([wq, wk], 0).T),
            "wv": bf(wv.T),
            "wout": bf(W_out[:, rs].T),
            "bqk": np.ascontiguousarray(
                np.concatenate([b_qkv[0 * D:1 * D][rs],
                                b_qkv[1 * D:2 * D][rs]])[:, None], np.float32),
            "bv": np.ascontiguousarray(b_qkv[2 * D:3 * D][rs][None, :], np.float32),
            "trid": bf(trid),
            "seld": np.concatenate([
                np.repeat(np.eye(2, dtype=np.float32), 64, axis=1)[0:1],
                np.zeros((31, 128), np.float32),
                np.repeat(np.eye(2, dtype=np.float32), 64, axis=1)[1:2],
                np.zeros((1, 128), np.float32)]),
        })
    return in_maps


def assemble_output(results, b_out):
    y = np.zeros((B, L, D), np.float32)
    for c in range(N_CORES):
        b = c // 4
        y[b] += np.asarray(results[c]["yt"], np.float32).T
    y += b_out[None, None, :].astype(np.float32)
    return y


def run(x, mask, W_qkv, b_qkv, W_out, b_out, trace=False, **spmd_kwargs):
    causal = np.array_equal(
        np.asarray(mask).reshape(L, L),
        np.triu(np.ones((L, L), bool), k=1))
    if not causal:
        # Fallback (never expected): reference semantics on host.
        print("WARNING: non-causal mask; computing on host")
        q, k, v = np.split(x @ W_qkv.T + b_qkv, 3, axis=-1)
        th = lambda t: t.reshape(B, L, H, HD).transpose(0, 2, 1, 3)
        q, k, v = th(q), th(k), th(v)
        a = np.einsum('bhqd,bhkd->bhqk', q, k) * SCALE
        a = np.where(np.asarray(mask), -np.inf, a)
        a = a - a.max(-1, keepdims=True)
        a = np.exp(a)
        a /= a.sum(-1, keepdims=True)
        o = np.einsum('bhqk,bhkd->bhqd', a, v)
        o = o.transpose(0, 2, 1, 3).reshape(B, L, D)
        return o @ W_out.T + b_out, None

    nc = _get_nc()
    in_maps = make_in_maps(np.asarray(x), np.asarray(W_qkv),
                           np.asarray(b_qkv), np.asarray(W_out))
    res = run_bass_kernel_spmd(nc, in_maps, list(range(N_CORES)),
                               trace=trace, **spmd_kwargs)
    y = assemble_output(res.results, np.asarray(b_out))
    return y, res


def kernel(x, mask, W_qkv, b_qkv, W_out, b_out):
    y, _ = run(x, mask, W_qkv, b_qkv, W_out, b_out)
    return y
